# revision 1
# baseline (speedup 1.0000x reference)
"""Trainium2 Bass kernel for nn_BoxIMFDGCNN (DGCNN-style dynamic-KNN GNN).

v3 strategy (8 NeuronCores, data-parallel over nodes):
  - Each core owns a 2048-node shard but embeds the FULL feature matrix
    (redundantly - cheaper than an AllGather in practice) plus its shard
    slice, all in exact fp32.
  - EdgeConv algebraic reduction: max_j leaky(MLP([x_i, x_j - x_i])) =
    leaky(A_i + max_j B_j) with A = x @ (W_top - W_bot), B = x @ W_bot + b.
  - KNN scores s_ij = x_i . y_j - 0.5|y_j|^2 computed exactly in fp32 on
    the PE (layer 1: K=128 main + two f32r K=1 norm rows; layer 2: K=65
    with the norm fused as contraction row 64 - exact fp32, no extra
    passes). DVE `max`/`max_index` extract top-8 per 1024-col chunk (the
    irreducible 2-pass scan); the 16x8 candidates merge via a 16-bit
    quantized score packed with the 14-bit column index into a sortable
    fp32 key. Rank 0 is self; ranks 1..10 are the KNN. All merge/pack ops
    and the K=10 batched indirect B-row gather run on GPSIMD so the DVE
    does nothing but scan. The tile loop is software-pipelined: tile t's
    scans overlap tile t-1's merge/gather/conv.
  - The g1 AllGather is split into AGC column-chunked AllGathers issued
    as soon as each group of tiles finishes (lagged so the collective
    never blocks a queue waiting on an eviction), overlapping L1;
    B2/norm2 are computed locally from the gathered g1. L2 streams its
    moving operand from the AllGather outputs (even chunks first, so
    the first scans never wait on the last AllGather), two tiles per
    block so each chunk is loaded once per block. finishA (merge ->
    gathers) and finishC (max-pool -> conv) trail the scans by one and
    two blocks respectively, keeping the DVE scan stream stall-free.
"""

import numpy as np

N = 16384
P = 128
NCORES = 8
SHARD = N // NCORES          # 2048
TILES = SHARD // P           # 16 row tiles per core
L = 1024                     # selection chunk width
NCHUNK = N // L              # 16
CAND = NCHUNK * 8            # 128 candidates per row
K = 10
HID = 128
DGC = 64
NCLS = 16
LEAK = 0.01
CW = 512
AGC = 4                      # g1 AllGather split into this many chunks
AGW = SHARD // AGC           # 512 columns per AG chunk

# Merge-key windows (raw-score units, host-derived with margins; values
# outside clamp to the window edges and can never be in the top-11).
SUB1, TOP1 = 0.0, 5.25
SUB2, TOP2 = 0.04, 0.70

_CACHE = {}
DEBUG = False
BATCHED_GATHER = False
# Dummy-input width: changes the HLO signature so stale executable caches
# (keyed without the embedded BIR payload) can never serve an old kernel.
BUILD_SALT = 5


def _build():
    import concourse.bass as bass
    import concourse.mybir as mybir
    import concourse.tile as tile
    from concourse import bacc
    from concourse.masks import make_identity

    f32 = mybir.dt.float32

    nc = bacc.Bacc("TRN2", target_bir_lowering=False, debug=False,
                   num_devices=NCORES)

    def din(name, shape):
        return nc.dram_tensor(name, shape, f32, kind="ExternalInput").ap()

    io = dict(
        nfT=din("nfT", [8, N]), rfT=din("rfT", [64, N]),
        txT=din("txT", [64, N]),
        nfTs=din("nfTs", [8, SHARD]), rfTs=din("rfTs", [64, SHARD]),
        txTs=din("txTs", [64, SHARD]),
        Wb=din("Wb", [8, 64]), Wr=din("Wr", [64, 64]), Wt=din("Wt", [64, 64]),
        bbT=din("bbT", [64, 1]), brT=din("brT", [64, 1]),
        btT=din("btT", [64, 1]),
        WfB=din("WfB", [64, HID]), WfR=din("WfR", [64, HID]),
        WfX=din("WfX", [64, HID]), bfT=din("bfT", [HID, 1]),
        W1a=din("W1a", [HID, DGC]), W1b=din("W1b", [HID, DGC]),
        be1=din("be1", [1, DGC]),
        W2a=din("W2a", [DGC, DGC]), W2b=din("W2b", [DGC, DGC]),
        be2=din("be2", [1, DGC]),
        WcA=din("WcA", [DGC, NCLS]), WcB=din("WcB", [DGC, NCLS]),
        bc=din("bc", [1, NCLS]),
        salt=din("salt", [1, BUILD_SALT]),
        out=nc.dram_tensor("out", [SHARD, NCLS], f32,
                           kind="ExternalOutput").ap(),
    )
    if DEBUG:
        import concourse.mybir as _mb
        for nm, shp, dt_ in [
            ("dbg_hT", [P, 2048], f32), ("dbg_norm1", [2, N], f32),
            ("dbg_B1", [N, DGC], f32), ("dbg_A1", [P, DGC], f32),
            ("dbg_nidx", [P, K], _mb.dt.uint32), ("dbg_M", [P, DGC], f32),
            ("dbg_g1T", [DGC, SHARD], f32), ("dbg_zT", [DGC + 1, N], f32),
            ("dbg_B2", [N, DGC], f32),
            ("dbg_nidx2", [P, K], _mb.dt.uint32), ("dbg_M2", [P, DGC], f32),
            ("dbg_g2T", [DGC, SHARD], f32),
        ]:
            io[nm] = nc.dram_tensor(nm, shp, dt_, kind="ExternalOutput").ap()

    with tile.TileContext(nc) as tc:
        _emit(nc, tc, bass, mybir, tile, make_identity, io)
    nc.compile()
    return nc


def _emit(nc, tc, bass, mybir, tile, make_identity, io):
    from contextlib import ExitStack
    from concourse.tile_rust import add_dep_helper as add_dep

    f32 = mybir.dt.float32
    f32r = mybir.dt.float32r
    u32 = mybir.dt.uint32
    Alu = mybir.AluOpType
    Act = mybir.ActivationFunctionType

    ctx = ExitStack()
    wpool = ctx.enter_context(tc.tile_pool(name="weights", bufs=1))
    inpool = ctx.enter_context(tc.tile_pool(name="inchunks", bufs=3))
    mpsum = ctx.enter_context(tc.tile_pool(name="mpsum", bufs=2, space="PSUM"))
    spsum = ctx.enter_context(tc.tile_pool(name="spsum", bufs=3, space="PSUM"))
    dram = ctx.enter_context(tc.tile_pool(name="dram", bufs=1, space="DRAM"))
    small = ctx.enter_context(tc.tile_pool(name="small", bufs=2))
    mid_pool = ctx.enter_context(tc.tile_pool(name="mid1", bufs=2))
    persist = ctx.enter_context(tc.tile_pool(name="persist", bufs=1))

    def wload(ap):
        t = wpool.tile(list(ap.shape), ap.dtype, name=f"w_{ap.tensor.name}")
        nc.scalar.dma_start(t[:], ap)
        return t

    saltt = wpool.tile([1, BUILD_SALT], f32, name="saltt")
    nc.sync.dma_start(saltt[:], io["salt"])
    w = {k: wload(io[k]) for k in
         ["Wb", "Wr", "Wt", "bbT", "brT", "btT", "WfB", "WfR", "WfX", "bfT",
          "W1a", "W1b", "be1", "W2a", "W2b", "be2", "WcA", "WcB", "bc"]}

    identity = wpool.tile([P, P], f32, name="identity")
    make_identity(nc, identity[:])
    ones_row = wpool.tile([1, P], f32, name="ones_row")
    nc.vector.memset(ones_row[:], 1.0)
    ones_row_r = wpool.tile([1, P], f32r, name="ones_row_r")
    nc.vector.tensor_copy(ones_row_r[:], ones_row[:])
    ones2 = wpool.tile([2, P], f32, name="ones2")
    nc.vector.memset(ones2[:], 1.0)
    ones2_r = wpool.tile([2, P], f32r, name="ones2_r")
    nc.vector.tensor_copy(ones2_r[:], ones2[:])
    zrow_r = wpool.tile([1, P], f32r, name="zrow_r")
    nc.vector.memset(zrow_r[:].bitcast(f32), 0.0)
    zrhs_r = wpool.tile([1, CW], f32r, name="zrhs_r")
    nc.vector.memset(zrhs_r[:].bitcast(f32), 0.0)
    ones_col = wpool.tile([P, 1], f32, name="ones_col")
    nc.vector.memset(ones_col[:], 1.0)
    cbase = wpool.tile([P, NCHUNK, 8], u32, name="cbase")
    nc.gpsimd.iota(cbase[:], pattern=[[L, NCHUNK], [0, 8]], base=0,
                   channel_multiplier=0)
    c14 = wpool.tile([P, 1], u32, name="c14")
    nc.vector.memset(c14[:], 14)
    cmask = wpool.tile([P, 1], u32, name="cmask")
    nc.vector.memset(cmask[:], 0x3FFF)

    def leaky(dst, src):
        nc.vector.scalar_tensor_tensor(dst, src, LEAK, src,
                                       op0=Alu.mult, op1=Alu.max)

    hTs = persist.tile([P, SHARD], f32, name="hTs")
    A1 = persist.tile([P, TILES, DGC], f32, name="A1")
    A2 = persist.tile([P, TILES, DGC], f32, name="A2")
    g1Tn = persist.tile([DGC + 1, SHARD], f32, name="g1Tn")
    nc.vector.memset(g1Tn[DGC:DGC + 1, :], 1.0)
    g2Ts = persist.tile([DGC, SHARD], f32, name="g2Ts")

    norm1_d = dram.tile([2, N], f32, name="norm1_d")
    B1 = dram.tile([N, DGC], f32, name="B1")
    B2 = dram.tile([N, DGC], f32, name="B2")
    ag_ins = [dram.tile([DGC + 1, AGW], f32, name=f"ag_in{i}")
              for i in range(AGC)]
    ag_outs = [dram.tile([NCORES * (DGC + 1), AGW], f32, name=f"ag_out{i}",
                         addr_space="Shared") for i in range(AGC)]

    def embed_chunk(epool, dst_ap, n_src, r_src, t_src, with_norm=None):
        """dst_ap [128, CW] <- leaky(Wf.T @ relu-embeds) for one col chunk."""
        nf_t = epool.tile([8, CW], f32, tag="nf")
        rf_t = epool.tile([64, CW], f32, tag="rf")
        tx_t = epool.tile([64, CW], f32, tag="tx")
        nc.sync.dma_start(nf_t[:], n_src)
        nc.sync.dma_start(rf_t[:], r_src)
        nc.sync.dma_start(tx_t[:], t_src)
        xb = epool.tile([64, CW], f32, tag="xb")
        xr = epool.tile([64, CW], f32, tag="xr")
        xt = epool.tile([64, CW], f32, tag="xt")
        for (src, wk, bk, dst) in [(nf_t, "Wb", "bbT", xb),
                                   (rf_t, "Wr", "brT", xr),
                                   (tx_t, "Wt", "btT", xt)]:
            ps = spsum.tile([64, CW], f32, tag="score", name="eps")
            nc.tensor.matmul(ps[:], w[wk][:], src[:], start=True, stop=True)
            nc.scalar.activation(dst[:], ps[:], Act.Relu, bias=w[bk][:, 0:1])
        ph = spsum.tile([P, CW], f32, tag="score", name="eph")
        nc.tensor.matmul(ph[:], w["WfB"][:], xb[:], start=True, stop=False)
        nc.tensor.matmul(ph[:], w["WfR"][:], xr[:], start=False, stop=False)
        nc.tensor.matmul(ph[:], w["WfX"][:], xt[:], start=False, stop=True)
        hpre = epool.tile([P, CW], f32, tag="hpre")
        nc.scalar.activation(hpre[:], ph[:], Act.Identity, bias=w["bfT"][:, 0:1])
        leaky(dst_ap, hpre[:])
        if with_norm is None:
            return
        # norm row: -0.5 * sum h^2, f32r hi/lo pair -> norm1_d columns
        sl = with_norm
        hsq = epool.tile([P, CW], f32, tag="hsq")
        nc.scalar.activation(hsq[:], dst_ap, Act.Square)
        psq = mpsum.tile([1, CW], f32, tag="m")
        nc.tensor.matmul(psq[:], ones_col[:], hsq[:], start=True, stop=True)
        nf32 = epool.tile([1, CW], f32, tag="nf32")
        nc.scalar.activation(nf32[:], psq[:], Act.Identity, scale=-0.5)
        nhi = epool.tile([1, CW], f32r, tag="nhi")
        nc.gpsimd.tensor_copy(nhi[:], nf32[:])
        dlo = epool.tile([1, CW], f32, tag="dlo")
        nc.gpsimd.tensor_tensor(dlo[:], nf32[:], nhi[:].bitcast(f32),
                                Alu.subtract)
        nlo = epool.tile([1, CW], f32r, tag="nlo")
        nc.gpsimd.tensor_copy(nlo[:], dlo[:])
        nc.sync.dma_start(norm1_d[0:1, sl].bitcast(f32r), nhi[:])
        nc.sync.dma_start(norm1_d[1:2, sl].bitcast(f32r), nlo[:])

    # ---------------- phase E: full embed + B1 + A1 + norms ----------------
    with tc.tile_pool(name="l1", bufs=1) as l1pool:
        hT = l1pool.tile([P, N], f32, name="hT")
        with tc.tile_pool(name="embed", bufs=2) as epool:
            for c in range(N // CW):
                sl = slice(c * CW, (c + 1) * CW)
                embed_chunk(epool, hT[:, sl], io["nfT"][:, sl],
                            io["rfT"][:, sl], io["txT"][:, sl], with_norm=sl)
            for c in range(SHARD // CW):
                sl = slice(c * CW, (c + 1) * CW)
                embed_chunk(epool, hTs[:, sl], io["nfTs"][:, sl],
                            io["rfTs"][:, sl], io["txTs"][:, sl])
            for t in range(N // P):
                tsl = slice(t * P, (t + 1) * P)
                pb = mpsum.tile([P, DGC], f32, tag="m")
                nc.tensor.matmul(pb[:], hT[:, tsl], w["W1b"][:], start=True,
                                 stop=True)
                bs = epool.tile([P, DGC], f32, tag="bev")
                nc.scalar.activation(bs[:], pb[:], Act.Identity)
                nc.sync.dma_start(B1[tsl, :], bs[:])
            for t in range(TILES):
                tsl = slice(t * P, (t + 1) * P)
                pa = mpsum.tile([P, DGC], f32, tag="m")
                nc.tensor.matmul(pa[:], hTs[:, tsl], w["W1a"][:], start=True,
                                 stop=False)
                nc.tensor.matmul(pa[:], ones_row[:], w["be1"][:], start=False,
                                 stop=True)
                nc.scalar.activation(A1[:, t], pa[:], Act.Identity)

        if DEBUG:
            nc.sync.dma_start(io["dbg_hT"], hT[:, 0:2048])
            nc.sync.dma_start(io["dbg_norm1"], norm1_d[:, :])
            nc.sync.dma_start(io["dbg_B1"], B1[:, :])
            nc.sync.dma_start(io["dbg_A1"], A1[:, 0])

        bprobe1 = inpool.tile([P, N // P], f32, tag="bprobe")
        fence1 = nc.sync.dma_start(
            bprobe1[:], B1[:, 0:1].rearrange("(a p) b -> p (a b)", p=P))

        # g1 AG chunk emission, interleaved into the L1 tile loop.
        # AG chunk i covers tiles {2i, 2i+1}; it is emitted at finish(2i+3)
        # so the tile-(2i+1) transpose+eviction has long cleared the PE
        # queue and the collective never holds the Pool SEQ waiting.
        # B2 for AG chunk j is emitted at finish(2j+5), after AG j is done.
        TPC = TILES // AGC           # tiles per AG chunk (2)

        def ag_emit(i, mid_pool):
            csl = slice(i * AGW, (i + 1) * AGW)
            gsq = mid_pool.tile([DGC, AGW], f32, tag="gsq")
            nc.scalar.activation(gsq[:], g1Tn[0:DGC, csl], Act.Square)
            psq = mpsum.tile([1, AGW], f32, tag="m")
            nc.tensor.matmul(psq[:], ones_col[0:DGC, :], gsq[:], start=True,
                             stop=True)
            n2 = mid_pool.tile([1, AGW], f32, tag="n2")
            nc.scalar.activation(n2[:], psq[:], Act.Identity, scale=-0.5)
            nc.sync.dma_start(ag_ins[i][DGC:DGC + 1, :], n2[:])
            nc.sync.dma_start(ag_ins[i][0:DGC, :], g1Tn[0:DGC, csl])
            nc.gpsimd.collective_compute(
                "AllGather", mybir.AluOpType.bypass,
                replica_groups=[list(range(NCORES))],
                ins=[ag_ins[i][:].opt()], outs=[ag_outs[i][:].opt()])

        def b2_emit(i, mid_pool):
            for cb in range(NCORES):
                zl = mid_pool.tile([DGC, AGW], f32, tag="zl")
                nc.scalar.dma_start(
                    zl[:], ag_outs[i][cb * (DGC + 1):cb * (DGC + 1) + DGC, :])
                for s in range(AGW // P):
                    j0 = cb * SHARD + i * AGW + s * P
                    pb = mpsum.tile([P, DGC], f32, tag="m")
                    nc.tensor.matmul(pb[:], zl[:, s * P:(s + 1) * P],
                                     w["W2b"][:], start=True, stop=True)
                    bs = mid_pool.tile([P, DGC], f32, tag="bev")
                    nc.scalar.activation(bs[:], pb[:], Act.Identity)
                    nc.scalar.dma_start(B2[j0:j0 + P, :], bs[:])

        def a2_emit(t):
            tsl = slice(t * P, (t + 1) * P)
            pa = mpsum.tile([P, DGC], f32, tag="m")
            nc.tensor.matmul(pa[:], g1Tn[0:DGC, tsl], w["W2a"][:],
                             start=True, stop=False)
            nc.tensor.matmul(pa[:], ones_row[:], w["be2"][:],
                             start=False, stop=True)
            nc.scalar.activation(A2[:, t], pa[:], Act.Identity)

        def post_tile1(t):
            a2_emit(t)
            if t >= TPC - 1 and (t - (TPC - 1)) % TPC == 0:
                ag_emit((t - (TPC - 1)) // TPC, mid_pool)
            if t >= 2 * TPC - 1 and (t - (2 * TPC - 1)) % TPC == 0:
                b2_emit((t - (2 * TPC - 1)) // TPC, mid_pool)

        _knn_layer(nc, bass, mybir, spsum, mpsum, small, inpool,
                   lhsT=hTs, kp=P, rhsT=hT, norm_d=norm1_d,
                   Btab=B1, A=A1, g_out_T=g1Tn[0:DGC, :],
                   identity=identity, ones2_r=ones2_r,
                   cbase=cbase, c14=c14, cmask=cmask, leaky=leaky,
                   fused_norm=False, sub=SUB1, zfill=None,
                   sc=65534.0 / (TOP1 - SUB1),
                   fence=fence1, add_dep=add_dep,
                   post_tile=post_tile1,
                   dbg=dict(nidx=io["dbg_nidx"], M=io["dbg_M"])
                   if DEBUG else None)
        # leftover AG / B2 chunks not covered inside the tile loop
        for i in range(AGC):
            if TPC - 1 + i * TPC > TILES - 1:
                ag_emit(i, mid_pool)
        for i in range(AGC):
            if 2 * TPC - 1 + i * TPC > TILES - 1:
                b2_emit(i, mid_pool)

    if DEBUG:
        nc.sync.dma_start(io["dbg_g1T"], g1Tn[0:DGC, :])

    if DEBUG:
        for cb in range(NCORES):
            for i in range(AGC):
                c0 = cb * SHARD + i * AGW
                nc.sync.dma_start(
                    io["dbg_zT"][:, c0:c0 + AGW],
                    ag_outs[i][cb * (DGC + 1):(cb + 1) * (DGC + 1), :])
        nc.sync.dma_start(io["dbg_B2"], B2[:, :])

    bprobe2 = inpool.tile([P, N // P], f32, tag="bprobe")
    fence2 = nc.scalar.dma_start(
        bprobe2[:], B2[:, 0:1].rearrange("(a p) b -> p (a b)", p=P))

    # ---------------- layer 2: KNN + EdgeConv + classifier -----------------
    def post_tile2(t):
        tsl = slice(t * P, (t + 1) * P)
        pl = mpsum.tile([P, NCLS], f32, tag="m")
        nc.tensor.matmul(pl[:], g1Tn[0:DGC, tsl], w["WcA"][:],
                         start=True, stop=False)
        nc.tensor.matmul(pl[:], g2Ts[:, tsl], w["WcB"][:], start=False,
                         stop=False)
        nc.tensor.matmul(pl[:], ones_row[:], w["bc"][:], start=False,
                         stop=True)
        lo = inpool.tile([P, NCLS], f32, tag="lo")
        nc.scalar.activation(lo[:], pl[:], Act.Identity)
        nc.scalar.dma_start(io["out"][tsl, :], lo[:])

    _knn_layer(nc, bass, mybir, spsum, mpsum, small, inpool,
               lhsT=g1Tn, kp=DGC + 1, rhsT=ag_outs, norm_d=None,
               Btab=B2, A=A2, g_out_T=g2Ts[:, :],
               identity=identity, ones2_r=ones2_r,
               cbase=cbase, c14=c14, cmask=cmask, leaky=leaky,
               fused_norm=True, sub=SUB2, sc=65534.0 / (TOP2 - SUB2),
               zfill=None,
               fence=fence2, add_dep=add_dep, rhs_dram=True, tb=2,
               post_tile=post_tile2,
               dbg=dict(nidx=io["dbg_nidx2"], M=io["dbg_M2"])
               if DEBUG else None)

    if DEBUG:
        nc.sync.dma_start(io["dbg_g2T"], g2Ts[:, :])

    ctx.close()


def _knn_layer(nc, bass, mybir, spsum, mpsum, small, inpool,
               lhsT, kp, rhsT, norm_d, Btab, A, g_out_T, identity,
               ones2_r, cbase, c14, cmask, leaky, fused_norm,
               sub, sc, fence=None, add_dep=None, post_tile=None,
               rhs_dram=False, tb=1, zfill=None, dbg=None):
    """One dynamic-KNN EdgeConv layer for this core's 2048-node shard.

    Software-pipelined over blocks of `tb` tiles: block b's chunk scans
    (chunk-major within the block; a streamed rhs chunk is loaded once
    per block) overlap block b-1's merge/gather/conv ops. The MaxIndex
    for a chunk is emitted after the next Max so the DVE dependency
    bubble between the two is hidden. When `rhs_dram` is set, rhsT is
    the list of per-AG-chunk DRAM outputs [NCORES*(DGC+1), AGW] and the
    [kp, L] moving operand is assembled from several of them per chunk.
    """
    f32 = mybir.dt.float32
    f32r = mybir.dt.float32r
    u32 = mybir.dt.uint32
    Alu = mybir.AluOpType
    Act = mybir.ActivationFunctionType

    state = {}

    def scans_block(b, injectA=None):
        ts = range(b * tb, (b + 1) * tb)
        cands = {}
        for t in ts:
            cands[t] = (small.tile([P, CAND], f32, tag="cval", bufs=4,
                                   name="cval"),
                        small.tile([P, CAND], u32, tag="cidx", bufs=4,
                                   name="cidx"))
        pending = []

        def flush_pending():
            for (pt, pc, pps) in pending:
                cval, cidx = cands[pt]
                nc.vector.max_index(out=cidx[:, pc * 8:(pc + 1) * 8],
                                    in_max=cval[:, pc * 8:(pc + 1) * 8],
                                    in_values=pps[:])
            pending.clear()

        corder = (list(range(0, NCHUNK, 2)) + list(range(1, NCHUNK, 2))
                  if rhs_dram else list(range(NCHUNK)))
        for ci, c in enumerate(corder):
            csl = slice(c * L, (c + 1) * L)
            if rhs_dram:
                zc = inpool.tile([kp, L], f32, tag="zc", bufs=2)
                cb, rem = divmod(c * L, SHARD)
                i0 = rem // AGW
                for q in range(L // AGW):
                    nc.sync.dma_start(
                        zc[:, q * AGW:(q + 1) * AGW],
                        rhsT[i0 + q][cb * (DGC + 1):cb * (DGC + 1) + kp, :])
                rsrc = zc
            else:
                rsrc = rhsT[0:kp, csl]
            if not fused_norm:
                nrm2 = inpool.tile([2, L], f32r, tag="nrm2", bufs=2)
                nc.sync.dma_start(nrm2[:], norm_d[:, csl].bitcast(f32r))
            for t in ts:
                lt = lhsT[0:kp, t * P:(t + 1) * P]
                cval, cidx = cands[t]
                ps = spsum.tile([P, L], f32, tag="score")
                nf = 0 if zfill is None else zfill[2]
                for h in range(2):
                    hs = slice(h * CW, (h + 1) * CW)
                    po = ps[:, hs]
                    last = nf == 0
                    if fused_norm:
                        nc.tensor.matmul(po, lt, rsrc[0:kp, hs],
                                         start=True, stop=last)
                    else:
                        nc.tensor.matmul(po, lt, rsrc[0:kp, hs],
                                         start=True, stop=False)
                        nc.tensor.matmul(po, ones2_r[:], nrm2[:, hs],
                                         start=False, stop=last)
                    # zero-contribution fillers: keep the PE the pacer so
                    # micro-stalls never reset the p-state ramp
                    for z in range(nf):
                        nc.tensor.matmul(po, zfill[0][:], zfill[1][:],
                                         start=False, stop=(z == nf - 1))
                nc.vector.max(out=cval[:, c * 8:(c + 1) * 8], in_=ps[:])
                pending.append((t, c, ps))
            if tb > 1 or c == NCHUNK - 1:
                flush_pending()
            elif c > 0:
                # lag-1 flush: all but the newest entry
                newest = pending.pop()
                flush_pending()
                pending.append(newest)
            if ci == 2 and injectA is not None:
                injectA()
        flush_pending()
        for t in ts:
            state[t] = cands[t]

    def finishA(t):
        """Merge candidates -> nidx -> launch B-row gathers (Pool)."""
        cval, cidx = state.pop(t)
        gidx = small.tile([P, CAND], u32, tag="gidx")
        nc.gpsimd.tensor_tensor(gidx[:], cidx[:],
                                cbase[:].rearrange("p a b -> p (a b)"),
                                Alu.add)
        qf = small.tile([P, CAND], f32, tag="qf")
        nc.gpsimd.tensor_scalar(qf[:], cval[:], sub, sc,
                                op0=Alu.subtract, op1=Alu.mult)
        nc.gpsimd.tensor_scalar(qf[:], qf[:], 1.0, 65535.0,
                                op0=Alu.max, op1=Alu.min)
        qu = small.tile([P, CAND], u32, tag="qu")
        nc.gpsimd.tensor_copy(qu[:], qf[:])              # f32 -> u32 trunc
        key = small.tile([P, CAND], u32, tag="key")
        nc.gpsimd.tensor_scalar(key[:], qu[:], 16384, None, op0=Alu.mult)
        nc.gpsimd.tensor_tensor(key[:], key[:], gidx[:], Alu.add)
        keyf = key[:].bitcast(f32)
        mk1 = small.tile([P, 8], f32, tag="mk1")
        nc.vector.max(out=mk1[:], in_=keyf)
        key2 = small.tile([P, CAND], f32, tag="key2")
        nc.vector.match_replace(out=key2[:], in_to_replace=mk1[:],
                                in_values=keyf, imm_value=0.0)
        mk2 = small.tile([P, 8], f32, tag="mk2")
        nc.vector.max(out=mk2[:], in_=key2[:])
        nidx = small.tile([P, K], u32, tag="nidx")
        nc.vector.tensor_tensor(nidx[:, 0:7], mk1[:, 1:8].bitcast(u32),
                                cmask[:].to_broadcast([P, 7]),
                                Alu.bitwise_and)
        nc.vector.tensor_tensor(nidx[:, 7:10], mk2[:, 0:3].bitcast(u32),
                                cmask[:].to_broadcast([P, 3]),
                                Alu.bitwise_and)
        if dbg is not None and t == 0:
            nc.sync.dma_start(dbg["nidx"], nidx[:])
        xj = small.tile([P, K, DGC], f32, tag="xj", bufs=3)
        for kk in range(K):
            gi = nc.gpsimd.indirect_dma_start(
                out=xj[:, kk, :], out_offset=None, in_=Btab[:, :],
                in_offset=bass.IndirectOffsetOnAxis(
                    ap=nidx[:, kk:kk + 1], axis=0))
            if fence is not None:
                add_dep(gi.ins, fence.ins,
                        reason="indirect gather waits for B table writes")
        gathered[t] = xj

    def finishC(t):
        """Max-pool over k, conv epilogue, transpose, evict."""
        xj = gathered.pop(t)
        M = small.tile([P, DGC], f32, tag="M")
        nc.vector.tensor_reduce(M[:], xj[:].rearrange("p k c -> p c k"),
                                axis=mybir.AxisListType.X, op=Alu.max)
        if dbg is not None and t == 0:
            nc.sync.dma_start(dbg["M"], M[:])
        pre = small.tile([P, DGC], f32, tag="pre")
        nc.vector.tensor_tensor(pre[:], A[:, t], M[:], Alu.add)
        lk = small.tile([P, DGC], f32, tag="lk")
        leaky(lk[:], pre[:])
        tp = mpsum.tile([DGC, P], f32, tag="m")
        nc.tensor.transpose(tp[:], lk[:], identity[:])
        nc.scalar.activation(g_out_T[:, t * P:(t + 1) * P], tp[:],
                             Act.Identity)
        if post_tile is not None:
            post_tile(t)

    gathered = {}
    nblocks = TILES // tb
    for b in range(nblocks):
        scans_block(b)
        if b > 0:
            for t in range((b - 1) * tb, b * tb):
                finishA(t)
        if b > 1:
            for t in range((b - 2) * tb, (b - 1) * tb):
                finishC(t)
    for t in range((nblocks - 1) * tb, TILES):
        finishA(t)
    for t in range((nblocks - 2) * tb, TILES):
        finishC(t)


def _prep_inputs(inputs):
    """Host-side: transpose features, shard, pre-arrange weights."""
    f = np.float32
    nf = np.ascontiguousarray(np.asarray(inputs["node_feat"]).T.astype(f))
    rf = np.ascontiguousarray(np.asarray(inputs["rf_feat"]).T.astype(f))
    tx = np.ascontiguousarray(np.asarray(inputs["txp_feat"]).T.astype(f))
    Wf = np.asarray(inputs["Wf"], f)
    We1 = np.asarray(inputs["We1"], f)
    We2 = np.asarray(inputs["We2"], f)
    Wc = np.asarray(inputs["Wc"], f)
    base = {
        "nfT": nf, "rfT": rf, "txT": tx,
        "Wb": np.asarray(inputs["Wb"], f),
        "Wr": np.asarray(inputs["Wr"], f),
        "Wt": np.asarray(inputs["Wt"], f),
        "bbT": np.asarray(inputs["bb"], f).reshape(64, 1).copy(),
        "brT": np.asarray(inputs["br"], f).reshape(64, 1).copy(),
        "btT": np.asarray(inputs["bt"], f).reshape(64, 1).copy(),
        "WfB": np.ascontiguousarray(Wf[0:64]),
        "WfR": np.ascontiguousarray(Wf[64:128]),
        "WfX": np.ascontiguousarray(Wf[128:192]),
        "bfT": np.asarray(inputs["bf"], f).reshape(HID, 1).copy(),
        "W1a": np.ascontiguousarray(We1[:HID] - We1[HID:]),
        "W1b": np.ascontiguousarray(We1[HID:]),
        "be1": np.asarray(inputs["be1"], f).reshape(1, DGC).copy(),
        "W2a": np.ascontiguousarray(We2[:DGC] - We2[DGC:]),
        "W2b": np.ascontiguousarray(We2[DGC:]),
        "be2": np.asarray(inputs["be2"], f).reshape(1, DGC).copy(),
        "WcA": np.ascontiguousarray(Wc[:DGC]),
        "WcB": np.ascontiguousarray(Wc[DGC:]),
        "bc": np.asarray(inputs["bc"], f).reshape(1, NCLS).copy(),
    }
    in_maps = []
    for c in range(NCORES):
        sl = slice(c * SHARD, (c + 1) * SHARD)
        m = dict(base)
        m["salt"] = np.zeros((1, BUILD_SALT), f)
        m["nfTs"] = np.ascontiguousarray(nf[:, sl])
        m["rfTs"] = np.ascontiguousarray(rf[:, sl])
        m["txTs"] = np.ascontiguousarray(tx[:, sl])
        in_maps.append(m)
    return in_maps


def kernel(**inputs):
    from concourse.bass_utils import run_bass_kernel_spmd

    if "nc" not in _CACHE:
        _CACHE["nc"] = _build()
    nc = _CACHE["nc"]
    in_maps = _prep_inputs(inputs)
    res = run_bass_kernel_spmd(nc, in_maps, core_ids=list(range(NCORES)))
    outs = [res.results[c]["out"] for c in range(NCORES)]
    return np.concatenate(outs, axis=0).astype(np.float32)


if __name__ == "__main__":
    import reference

    ins = {k: np.asarray(v) for k, v in reference.setup_inputs().items()}
    got = kernel(**ins)
    exp = np.asarray(reference.reference(**ins))
    err = np.abs(got - exp)
    print("max abs err:", err.max(), "rel:", err.max() / np.abs(exp).max())



# revision 23
# speedup vs baseline: 1.0159x; 1.0159x over previous
"""Trainium2 Bass kernel for nn_BoxIMFDGCNN (DGCNN-style dynamic-KNN GNN).

v4 strategy (8 NeuronCores, data-parallel over nodes):
  - All PE matmuls run in float32r (11-bit-mantissa fp32, 1 cycle/row for
    free-dim >= 256, vs 4 for fp32): score matrices, embeds, fusion and
    tables. End-to-end precision emulation puts the resulting L2 error at
    ~0.008, inside the 2e-2 gate.
  - EdgeConv algebraic reduction: max_j leaky(MLP([x_i, x_j - x_i])) =
    leaky(A_i + max_j B_j) with A = x @ (W_top - W_bot), B = x @ W_bot + b.
  - KNN scores s_ij = x_i . y_j - 0.5|y_j|^2 on the PE (layer 1: K=128 main
    + two f32r hi/lo norm rows; layer 2: K=65 with the norm fused as
    contraction row 64). DVE max/max_index extract top-8 per 1024-col chunk
    (the 2-pass scan is the kernel's floor: ~1.04 ns/elem/pass, no dtype or
    perf-mode discount exists for InstMax). Candidates merge via a 16-bit
    quantized score packed with the 14-bit column index into a sortable
    fp32 key; merge/pack runs on GPSIMD, B-row gathers are a single
    batched K=10 indirect DMA per tile.
  - The embed phase is interleaved into the first L1 scan block (tb=2
    tiles/block) so the DVE starts scanning ~5us into the kernel: the
    pre_chunk hook embeds scan-chunk c+LOOKAHEAD (plus its B1 rows and
    norm rows) right after the scan matmuls of chunk c are emitted.
  - g1 AllGather is split into 4 column chunks issued as tiles complete,
    overlapping L1. The gathered z (g1 + norm row) is staged once into a
    persistent SBUF tile [65, N]; layer 2 streams its moving operand from
    SBUF (no per-chunk DRAM reloads) and B2 is computed from the same
    tile. The classifier is fused into layer 2's finishC.
"""

import numpy as np

N = 16384
P = 128
NCORES = 8
SHARD = N // NCORES          # 2048
TILES = SHARD // P           # 16 row tiles per core
L = 1024                     # selection chunk width
NCHUNK = N // L              # 16
CAND = NCHUNK * 8            # 128 candidates per row
K = 10
HID = 128
DGC = 64
NCLS = 16
LEAK = 0.01
CW = 512
BLOCKS1 = [3, 3, 2, 2, 2, 2, 2]   # L1 tile-block sizes (block 0 hides embeds)
BLOCKS2 = [2] * 8                 # L2 tile-block sizes
AGC = 4                      # g1 AllGather split into this many chunks
AGW = SHARD // AGC           # 512 columns per AG chunk
TPC = TILES // AGC           # tiles per AG chunk (4)
LOOKAHEAD = 2                # embed chunks emitted ahead of scans

# Merge-key windows (raw-score units, host-derived with margins; values
# outside clamp to the window edges and can never be in the top-11).
SUB1, TOP1 = 0.0, 5.25
SUB2, TOP2 = 0.04, 0.70

_CACHE = {}
DEBUG = False
# Dummy-input width: changes the HLO signature so stale executable caches
# (keyed without the embedded BIR payload) can never serve an old kernel.
BUILD_SALT = 6


def _build():
    import concourse.bass as bass
    import concourse.mybir as mybir
    import concourse.tile as tile
    from concourse import bacc
    from concourse.masks import make_identity

    f32 = mybir.dt.float32

    nc = bacc.Bacc("TRN2", target_bir_lowering=False, debug=False,
                   num_devices=NCORES)

    def din(name, shape):
        return nc.dram_tensor(name, shape, f32, kind="ExternalInput").ap()

    io = dict(
        nfT=din("nfT", [8, N]), rfT=din("rfT", [64, N]),
        txT=din("txT", [64, N]),
        nfTs=din("nfTs", [8, SHARD]), rfTs=din("rfTs", [64, SHARD]),
        txTs=din("txTs", [64, SHARD]),
        Wb=din("Wb", [8, 64]), Wr=din("Wr", [64, 64]), Wt=din("Wt", [64, 64]),
        bbT=din("bbT", [64, 1]), brT=din("brT", [64, 1]),
        btT=din("btT", [64, 1]),
        WfB=din("WfB", [64, HID]), WfR=din("WfR", [64, HID]),
        WfX=din("WfX", [64, HID]), bfT=din("bfT", [HID, 1]),
        W1a=din("W1a", [HID, DGC]), W1b=din("W1b", [HID, DGC]),
        be1=din("be1", [1, DGC]),
        W2a=din("W2a", [DGC, DGC]), W2b=din("W2b", [DGC, DGC]),
        be2=din("be2", [1, DGC]),
        WcA=din("WcA", [DGC, NCLS]), WcB=din("WcB", [DGC, NCLS]),
        bc=din("bc", [1, NCLS]),
        salt=din("salt", [1, BUILD_SALT]),
        out=nc.dram_tensor("out", [SHARD, NCLS], f32,
                           kind="ExternalOutput").ap(),
    )
    if DEBUG:
        import concourse.mybir as _mb
        for nm, shp, dt_ in [
            ("dbg_hT", [128, 2048], f32), ("dbg_norm1", [2, N], f32),
            ("dbg_B1", [N, DGC], f32), ("dbg_A1", [128, DGC], f32),
            ("dbg_nidx", [128, K], _mb.dt.uint32),
            ("dbg_xj", [128, K, DGC], f32),
            ("dbg_M", [128, DGC], f32), ("dbg_g1T", [DGC, SHARD], f32),
            ("dbg_B2", [N, DGC], f32),
            ("dbg_nidx2", [128, K], _mb.dt.uint32),
        ]:
            io[nm] = nc.dram_tensor(nm, shp, dt_, kind="ExternalOutput").ap()

    with tile.TileContext(nc) as tc:
        _emit(nc, tc, bass, mybir, tile, make_identity, io)
    nc.compile()
    return nc


def _emit(nc, tc, bass, mybir, tile, make_identity, io):
    from contextlib import ExitStack
    from concourse.tile_rust import add_dep_helper as add_dep

    f32 = mybir.dt.float32
    f32r = mybir.dt.float32r
    u32 = mybir.dt.uint32
    Alu = mybir.AluOpType
    Act = mybir.ActivationFunctionType

    ctx = ExitStack()
    wpool = ctx.enter_context(tc.tile_pool(name="weights", bufs=1))
    epool = ctx.enter_context(tc.tile_pool(name="embed", bufs=2))
    inpool = ctx.enter_context(tc.tile_pool(name="inchunks", bufs=2))
    mpsum = ctx.enter_context(tc.tile_pool(name="mpsum", bufs=2, space="PSUM"))
    spsum = ctx.enter_context(tc.tile_pool(name="spsum", bufs=3, space="PSUM"))
    dram = ctx.enter_context(tc.tile_pool(name="dram", bufs=1, space="DRAM"))
    small = ctx.enter_context(tc.tile_pool(name="small", bufs=2))
    mid_pool = ctx.enter_context(tc.tile_pool(name="mid1", bufs=2))
    persist = ctx.enter_context(tc.tile_pool(name="persist", bufs=1))

    def wload(ap, dt=f32r):
        t = wpool.tile(list(ap.shape), dt, name=f"w_{ap.tensor.name}")
        nc.scalar.dma_start(t[:], ap.bitcast(dt) if dt is f32r else ap)
        return t

    saltt = wpool.tile([1, BUILD_SALT], f32, name="saltt")
    nc.sync.dma_start(saltt[:], io["salt"])
    w = {k: wload(io[k]) for k in
         ["Wb", "Wr", "Wt", "WfB", "WfR", "WfX",
          "W1a", "W1b", "be1", "W2a", "W2b", "be2", "WcA", "WcB", "bc"]}
    for k in ["bbT", "brT", "btT", "bfT"]:
        w[k] = wload(io[k], f32)

    identity = wpool.tile([P, P], f32, name="identity")
    make_identity(nc, identity[:])
    ones_row = wpool.tile([1, P], f32, name="ones_row")
    nc.vector.memset(ones_row[:], 1.0)
    ones_row_r = wpool.tile([1, P], f32r, name="ones_row_r")
    nc.vector.tensor_copy(ones_row_r[:], ones_row[:])
    ones2 = wpool.tile([2, P], f32, name="ones2")
    nc.vector.memset(ones2[:], 1.0)
    ones2_r = wpool.tile([2, P], f32r, name="ones2_r")
    nc.vector.tensor_copy(ones2_r[:], ones2[:])
    ones_col = wpool.tile([P, 1], f32, name="ones_col")
    nc.vector.memset(ones_col[:], 1.0)
    ones_col_r = wpool.tile([P, 1], f32r, name="ones_col_r")
    nc.vector.tensor_copy(ones_col_r[:], ones_col[:])
    cbase = wpool.tile([P, NCHUNK, 8], u32, name="cbase")
    nc.gpsimd.iota(cbase[:], pattern=[[L, NCHUNK], [0, 8]], base=0,
                   channel_multiplier=0)
    cmask = wpool.tile([P, 1], u32, name="cmask")
    nc.vector.memset(cmask[:], 0x3FFF)

    hTs = persist.tile([P, SHARD], f32r, name="hTs")
    A1 = persist.tile([P, TILES, DGC], f32, name="A1")
    A2 = persist.tile([P, TILES, DGC], f32, name="A2")
    g1Tn = persist.tile([DGC + 1, SHARD], f32r, name="g1Tn")
    nc.gpsimd.memset(g1Tn[DGC:DGC + 1, :].bitcast(f32), 1.0)

    norm1_d = dram.tile([2, N], f32, name="norm1_d")
    B1 = dram.tile([N, DGC], f32, name="B1")
    B2 = dram.tile([N, DGC], f32, name="B2")
    ag_ins = [dram.tile([DGC + 1, AGW], f32, name=f"ag_in{i}")
              for i in range(AGC)]
    ag_outs = [dram.tile([NCORES * (DGC + 1), AGW], f32, name=f"ag_out{i}",
                         addr_space="Shared") for i in range(AGC)]

    def embed_chunk(dst_ap, n_src, r_src, t_src, with_norm=None):
        """dst_ap [128, CW] f32r <- leaky(Wf.T @ relu-embeds), one col chunk."""
        nf_t = epool.tile([8, CW], f32r, tag="nf")
        rf_t = epool.tile([64, CW], f32r, tag="rf")
        tx_t = epool.tile([64, CW], f32r, tag="tx")
        nc.sync.dma_start(nf_t[:], n_src.bitcast(f32r))
        nc.sync.dma_start(rf_t[:], r_src.bitcast(f32r))
        nc.sync.dma_start(tx_t[:], t_src.bitcast(f32r))
        xb = epool.tile([64, CW], f32r, tag="xb")
        xr = epool.tile([64, CW], f32r, tag="xr")
        xt = epool.tile([64, CW], f32r, tag="xt")
        for (src, wk, bk, dst) in [(nf_t, "Wb", "bbT", xb),
                                   (rf_t, "Wr", "brT", xr),
                                   (tx_t, "Wt", "btT", xt)]:
            ps = mpsum.tile([64, CW], f32, tag="m", name="eps")
            nc.tensor.matmul(ps[:], w[wk][:], src[:], start=True, stop=True)
            nc.scalar.activation(dst[:], ps[:], Act.Relu, bias=w[bk][:, 0:1])
        ph = mpsum.tile([P, CW], f32, tag="m", name="eph")
        nc.tensor.matmul(ph[:], w["WfB"][:], xb[:], start=True, stop=False)
        nc.tensor.matmul(ph[:], w["WfR"][:], xr[:], start=False, stop=False)
        nc.tensor.matmul(ph[:], w["WfX"][:], xt[:], start=False, stop=True)
        # fused bias + leaky-relu, PSUM -> f32r SBUF in one act op
        nc.scalar.activation(dst_ap, ph[:], Act.Lrelu, bias=w["bfT"][:, 0:1])
        if with_norm is None:
            return
        # norm row: -0.5 * sum h^2 as an f32r hi/lo pair -> norm1_d columns
        sl = with_norm
        hsq = epool.tile([P, CW], f32r, tag="hsq")
        nc.gpsimd.tensor_tensor(hsq[:], dst_ap.bitcast(f32),
                                dst_ap.bitcast(f32), Alu.mult)
        psq = mpsum.tile([1, CW], f32, tag="m")
        nc.tensor.matmul(psq[:], ones_col_r[:], hsq[:], start=True, stop=True)
        nf32 = epool.tile([1, CW], f32, tag="nf32")
        nc.scalar.activation(nf32[:], psq[:], Act.Identity, scale=-0.5)
        nhi = epool.tile([1, CW], f32r, tag="nhi")
        nc.gpsimd.tensor_copy(nhi[:], nf32[:])
        dlo = epool.tile([1, CW], f32, tag="dlo")
        nc.gpsimd.tensor_tensor(dlo[:], nf32[:], nhi[:].bitcast(f32),
                                Alu.subtract)
        nlo = epool.tile([1, CW], f32r, tag="nlo")
        nc.gpsimd.tensor_copy(nlo[:], dlo[:])
        nc.sync.dma_start(norm1_d[0:1, sl].bitcast(f32r), nhi[:])
        nc.sync.dma_start(norm1_d[1:2, sl].bitcast(f32r), nlo[:])

    def b_rows(Btab, Wkey, lhs_tile, c0):
        """Emit B rows for cols [c0, c0+1024) of lhs (8 row-tiles, batched)."""
        for half in range(2):
            pb = mpsum.tile([P, 4, DGC], f32, tag="m", name="pb")
            for s in range(4):
                tsl = slice(c0 + half * 512 + s * P,
                            c0 + half * 512 + (s + 1) * P)
                nc.tensor.matmul(pb[:, s, :], lhs_tile[:, tsl], w[Wkey][:],
                                 start=True, stop=True)
            bs = mid_pool.tile([P, 4, DGC], f32, tag="bev")
            nc.scalar.activation(bs[:], pb[:], Act.Identity)
            j0 = c0 + half * 512
            nc.scalar.dma_start(
                Btab[j0:j0 + 512, :].rearrange("(a p) b -> p a b", p=P),
                bs[:])

    # ---------------- phase E head: shard embed + A1 ----------------
    with tc.tile_pool(name="l1", bufs=1) as l1pool:
        hT = l1pool.tile([P, N], f32r, name="hT")
        for c in range(SHARD // CW):
            sl = slice(c * CW, (c + 1) * CW)
            embed_chunk(hTs[:, sl], io["nfTs"][:, sl], io["rfTs"][:, sl],
                        io["txTs"][:, sl])
        for t in range(TILES):
            tsl = slice(t * P, (t + 1) * P)
            pa = mpsum.tile([P, DGC], f32, tag="m")
            nc.tensor.matmul(pa[:], hTs[:, tsl], w["W1a"][:], start=True,
                             stop=False)
            nc.tensor.matmul(pa[:], ones_row_r[:], w["be1"][:], start=False,
                             stop=True)
            nc.scalar.activation(A1[:, t], pa[:], Act.Identity)

        embedded = [False] * NCHUNK

        def embed_scan_chunk(c):
            if embedded[c]:
                return
            embedded[c] = True
            for half in range(2):
                sl = slice(c * L + half * CW, c * L + (half + 1) * CW)
                embed_chunk(hT[:, sl], io["nfT"][:, sl], io["rfT"][:, sl],
                            io["txT"][:, sl], with_norm=sl)
            b_rows(B1, "W1b", hT, c * L)

        for c in range(LOOKAHEAD):
            embed_scan_chunk(c)

        def pre_chunk1(ci, c):
            nxt = ci + LOOKAHEAD
            if nxt < NCHUNK:
                embed_scan_chunk(nxt)

        fence_box1 = {}

        def post_block1(b):
            if b == 0:
                for c in range(NCHUNK):
                    embed_scan_chunk(c)
                bprobe = inpool.tile([P, N // P], f32, tag="bprobe")
                fence_box1["fence"] = nc.sync.dma_start(
                    bprobe[:], B1[:, 0:1].rearrange("(a p) b -> p (a b)", p=P))

        # g1 AG chunk i (g1 cols [i*AGW,(i+1)*AGW) = tiles 4i..4i+3) is
        # emitted at finishC(4i+3); its z unload + B2 rows at finishC(4i+7).
        def ag_emit(i):
            csl = slice(i * AGW, (i + 1) * AGW)
            gsq = mid_pool.tile([DGC, AGW], f32r, tag="gsq")
            nc.scalar.activation(gsq[:], g1Tn[0:DGC, csl].bitcast(f32),
                                 Act.Square)
            psq = mpsum.tile([1, AGW], f32, tag="m")
            nc.tensor.matmul(psq[:], ones_col_r[0:DGC, :], gsq[:], start=True,
                             stop=True)
            n2 = mid_pool.tile([1, AGW], f32r, tag="n2")
            nc.scalar.activation(n2[:], psq[:], Act.Identity, scale=-0.5)
            nc.sync.dma_start(ag_ins[i][DGC:DGC + 1, :].bitcast(f32r), n2[:])
            nc.sync.dma_start(ag_ins[i][0:DGC, :].bitcast(f32r),
                              g1Tn[0:DGC, csl])
            nc.gpsimd.collective_compute(
                "AllGather", mybir.AluOpType.bypass,
                replica_groups=[list(range(NCORES))],
                ins=[ag_ins[i][:].opt()], outs=[ag_outs[i][:].opt()])

        def z_b2_emit(i):
            for cb in range(NCORES):
                zl = mid_pool.tile([DGC, AGW], f32r, tag="zl")
                nc.scalar.dma_start(
                    zl[:], ag_outs[i][cb * (DGC + 1):cb * (DGC + 1) + DGC, :]
                    .bitcast(f32r))
                c0 = cb * SHARD + i * AGW
                pb = mpsum.tile([P, 4, DGC], f32, tag="m", name="pb")
                for s in range(4):
                    nc.tensor.matmul(pb[:, s, :], zl[:, s * P:(s + 1) * P],
                                     w["W2b"][:], start=True, stop=True)
                bs = mid_pool.tile([P, 4, DGC], f32, tag="bev")
                nc.scalar.activation(bs[:], pb[:], Act.Identity)
                nc.scalar.dma_start(
                    B2[c0:c0 + 512, :].rearrange("(a p) b -> p a b", p=P),
                    bs[:])

        def a2_emit(t):
            tsl = slice(t * P, (t + 1) * P)
            pa = mpsum.tile([P, DGC], f32, tag="m")
            nc.tensor.matmul(pa[:], g1Tn[0:DGC, tsl], w["W2a"][:],
                             start=True, stop=False)
            nc.tensor.matmul(pa[:], ones_row_r[:], w["be2"][:],
                             start=False, stop=True)
            nc.scalar.activation(A2[:, t], pa[:], Act.Identity)

        def emit_out1(t, lk_t):
            tp = mpsum.tile([DGC, P], f32, tag="m")
            nc.tensor.transpose(tp[:], lk_t[:], identity[:])
            nc.scalar.activation(g1Tn[0:DGC, t * P:(t + 1) * P], tp[:],
                                 Act.Identity)
            a2_emit(t)
            if t % TPC == TPC - 1:
                ag_emit(t // TPC)
            if t >= 2 * TPC - 1 and (t - (2 * TPC - 1)) % TPC == 0:
                z_b2_emit((t - (2 * TPC - 1)) // TPC)

        _knn_layer(nc, bass, mybir, spsum, mpsum, small, inpool,
                   lhsT=hTs, kp=P, rhsT=hT, norm_d=norm1_d,
                   Btab=B1, A=A1, identity=identity, ones2_r=ones2_r,
                   cbase=cbase, cmask=cmask, blocks=BLOCKS1,
                   fused_norm=False, sub=SUB1,
                   sc=65534.0 / (TOP1 - SUB1),
                   fence_box=fence_box1, add_dep=add_dep,
                   pre_chunk=pre_chunk1, post_block=post_block1,
                   emit_out=emit_out1,
                   dbg=dict(nidx=io["dbg_nidx"], xj=io["dbg_xj"],
                            M=io["dbg_M"]) if DEBUG else None)
        for i in range(AGC):
            if 2 * TPC - 1 + i * TPC > TILES - 1:
                z_b2_emit(i)
        if DEBUG:
            nc.sync.dma_start(io["dbg_hT"], hT[:, 0:2048].bitcast(f32))
            nc.sync.dma_start(io["dbg_norm1"], norm1_d[:, :])
            nc.sync.dma_start(io["dbg_B1"], B1[:, :])
            nc.sync.dma_start(io["dbg_A1"], A1[:, 0])
            nc.sync.dma_start(io["dbg_g1T"], g1Tn[0:DGC, :].bitcast(f32))

    if DEBUG:
        nc.sync.dma_start(io["dbg_B2"], B2[:, :])
    bprobe2 = inpool.tile([P, N // P], f32, tag="bprobe")
    fence2 = nc.scalar.dma_start(
        bprobe2[:], B2[:, 0:1].rearrange("(a p) b -> p (a b)", p=P))

    # ---------------- layer 2: KNN + EdgeConv + classifier -----------------
    def emit_out2(t, lk_t):
        tp = mpsum.tile([DGC, P], f32, tag="m")
        nc.tensor.transpose(tp[:], lk_t[:], identity[:])
        gt = small.tile([DGC, P], f32r, tag="gt")
        nc.scalar.activation(gt[:], tp[:], Act.Identity)
        tsl = slice(t * P, (t + 1) * P)
        pl = mpsum.tile([P, NCLS], f32, tag="m")
        nc.tensor.matmul(pl[:], g1Tn[0:DGC, tsl], w["WcA"][:],
                         start=True, stop=False)
        nc.tensor.matmul(pl[:], gt[:], w["WcB"][:], start=False, stop=False)
        nc.tensor.matmul(pl[:], ones_row_r[:], w["bc"][:], start=False,
                         stop=True)
        lo = inpool.tile([P, NCLS], f32, tag="lo")
        nc.scalar.activation(lo[:], pl[:], Act.Identity)
        nc.scalar.dma_start(io["out"][tsl, :], lo[:])

    _knn_layer(nc, bass, mybir, spsum, mpsum, small, inpool,
               lhsT=g1Tn, kp=DGC + 1, rhsT=ag_outs, norm_d=None,
               Btab=B2, A=A2, identity=identity, ones2_r=ones2_r,
               cbase=cbase, cmask=cmask, blocks=BLOCKS2,
               fused_norm=True, sub=SUB2, sc=65534.0 / (TOP2 - SUB2),
               fence_box={"fence": fence2}, add_dep=add_dep,
               emit_out=emit_out2, evens_first=True, rhs_dram=True,
               dbg=dict(nidx=io["dbg_nidx2"]) if DEBUG else None)

    ctx.close()


def _knn_layer(nc, bass, mybir, spsum, mpsum, small, inpool,
               lhsT, kp, rhsT, norm_d, Btab, A, identity,
               ones2_r, cbase, cmask, blocks, fused_norm,
               sub, sc, fence_box, add_dep, emit_out,
               pre_chunk=None, post_block=None, evens_first=False,
               rhs_dram=False, dbg=None):
    """One dynamic-KNN EdgeConv layer for this core's 2048-node shard.

    Software-pipelined over tile blocks: block b's chunk scans overlap
    block b-1's merge/gather and block b-2's conv epilogue. The MaxIndex
    for a chunk is emitted after all the chunk's Max ops so the DVE
    dependency bubble between the two is hidden.
    """
    f32 = mybir.dt.float32
    f32r = mybir.dt.float32r
    u32 = mybir.dt.uint32
    Alu = mybir.AluOpType
    Act = mybir.ActivationFunctionType

    state = {}
    gathered = {}

    def scans_block(b, ts):
        cands = {}
        for t in ts:
            cands[t] = (small.tile([P, CAND], f32, tag="cval", bufs=6,
                                   name="cval"),
                        small.tile([P, CAND], u32, tag="cidx", bufs=6,
                                   name="cidx"))
        corder = (list(range(0, NCHUNK, 2)) + list(range(1, NCHUNK, 2))
                  if evens_first else list(range(NCHUNK)))
        for ci, c in enumerate(corder):
            if pre_chunk is not None and b == 0:
                pre_chunk(ci, c)
            csl = slice(c * L, (c + 1) * L)
            if rhs_dram:
                zc = inpool.tile([kp, L], f32r, tag="zc", bufs=2)
                cb, rem = divmod(c * L, SHARD)
                i0 = rem // AGW
                for q in range(L // AGW):
                    nc.sync.dma_start(
                        zc[:, q * AGW:(q + 1) * AGW],
                        rhsT[i0 + q][cb * (DGC + 1):cb * (DGC + 1) + kp, :]
                        .bitcast(f32r))
                rsrc = zc[:]
            else:
                rsrc = rhsT[0:kp, csl]
            if not fused_norm:
                nrm2 = inpool.tile([2, L], f32r, tag="nrm2", bufs=2)
                nc.sync.dma_start(nrm2[:], norm_d[:, csl].bitcast(f32r))
            pending = []
            for t in ts:
                lt = lhsT[0:kp, t * P:(t + 1) * P]
                cval, cidx = cands[t]
                ps = spsum.tile([P, L], f32, tag="score")
                for h in range(2):
                    hs = slice(h * CW, (h + 1) * CW)
                    po = ps[:, hs]
                    if fused_norm:
                        nc.tensor.matmul(po, lt, rsrc[0:kp, hs],
                                         start=True, stop=True)
                    else:
                        nc.tensor.matmul(po, lt, rsrc[0:kp, hs],
                                         start=True, stop=False)
                        nc.tensor.matmul(po, ones2_r[:], nrm2[:, hs],
                                         start=False, stop=True)
                nc.vector.max(out=cval[:, c * 8:(c + 1) * 8], in_=ps[:])
                pending.append((t, c, ps))
            for (pt, pc, pps) in pending:
                cval, cidx = cands[pt]
                nc.vector.max_index(out=cidx[:, pc * 8:(pc + 1) * 8],
                                    in_max=cval[:, pc * 8:(pc + 1) * 8],
                                    in_values=pps[:])
        for t in ts:
            state[t] = cands[t]

    def finishA(t):
        """Merge candidates -> nidx -> batched B-row gather (Pool)."""
        cval, cidx = state.pop(t)
        gidx = small.tile([P, CAND], u32, tag="gidx")
        nc.gpsimd.tensor_tensor(gidx[:], cidx[:],
                                cbase[:].rearrange("p a b -> p (a b)"),
                                Alu.add)
        qf = small.tile([P, CAND], f32, tag="qf")
        nc.gpsimd.tensor_scalar(qf[:], cval[:], sub, sc,
                                op0=Alu.subtract, op1=Alu.mult)
        nc.gpsimd.tensor_scalar(qf[:], qf[:], 1.0, 65535.0,
                                op0=Alu.max, op1=Alu.min)
        qu = small.tile([P, CAND], u32, tag="qu")
        nc.gpsimd.tensor_copy(qu[:], qf[:])              # f32 -> u32 trunc
        key = small.tile([P, CAND], u32, tag="key")
        nc.gpsimd.tensor_scalar(key[:], qu[:], 16384, None, op0=Alu.mult)
        nc.gpsimd.tensor_tensor(key[:], key[:], gidx[:], Alu.add)
        keyf = key[:].bitcast(f32)
        mk1 = small.tile([P, 8], f32, tag="mk1")
        nc.vector.max(out=mk1[:], in_=keyf)
        key2 = small.tile([P, CAND], f32, tag="key2")
        nc.vector.match_replace(out=key2[:], in_to_replace=mk1[:],
                                in_values=keyf, imm_value=0.0)
        mk2 = small.tile([P, 8], f32, tag="mk2")
        nc.vector.max(out=mk2[:], in_=key2[:])
        nidx = small.tile([P, K], u32, tag="nidx")
        nc.vector.tensor_tensor(nidx[:, 0:7], mk1[:, 1:8].bitcast(u32),
                                cmask[:].to_broadcast([P, 7]),
                                Alu.bitwise_and)
        nc.vector.tensor_tensor(nidx[:, 7:10], mk2[:, 0:3].bitcast(u32),
                                cmask[:].to_broadcast([P, 3]),
                                Alu.bitwise_and)
        if dbg is not None and t == 0:
            nc.sync.dma_start(dbg["nidx"], nidx[:])
        xj = small.tile([P, K, DGC], f32, tag="xj", bufs=3)
        fence = fence_box.get("fence")
        for kk in range(K):
            gi = nc.gpsimd.indirect_dma_start(
                out=xj[:, kk, :], out_offset=None, in_=Btab[:, :],
                in_offset=bass.IndirectOffsetOnAxis(
                    ap=nidx[:, kk:kk + 1], axis=0))
            if fence is not None:
                add_dep(gi.ins, fence.ins,
                        reason="indirect gather waits for B table writes")
        gathered[t] = xj

    def finishC(t):
        """Max-pool over k, conv epilogue, output emission."""
        xj = gathered.pop(t)
        if dbg is not None and "xj" in dbg and t == 0:
            nc.sync.dma_start(dbg["xj"], xj[:])
        M = small.tile([P, DGC], f32, tag="M")
        nc.vector.tensor_reduce(M[:], xj[:].rearrange("p k c -> p c k"),
                                axis=mybir.AxisListType.X, op=Alu.max)
        if dbg is not None and "M" in dbg and t == 0:
            nc.sync.dma_start(dbg["M"], M[:])
        pre = small.tile([P, DGC], f32, tag="pre")
        nc.gpsimd.tensor_tensor(pre[:], A[:, t], M[:], Alu.add)
        lk = small.tile([P, DGC], f32, tag="lk")
        nc.scalar.activation(lk[:], pre[:], Act.Lrelu)
        emit_out(t, lk)

    starts = [sum(blocks[:i]) for i in range(len(blocks))]
    ranges = [range(s, s + n) for s, n in zip(starts, blocks)]
    for b, ts in enumerate(ranges):
        scans_block(b, ts)
        if post_block is not None:
            post_block(b)
        if b > 0:
            for t in ranges[b - 1]:
                finishA(t)
        if b > 1:
            for t in ranges[b - 2]:
                finishC(t)
    for t in ranges[-1]:
        finishA(t)
    for t in ranges[-2]:
        finishC(t)
    for t in ranges[-1]:
        finishC(t)


def _rnd_f32r(x):
    """RNE to 11 kept mantissa bits — the f32r grid measured on TRN2."""
    x = np.ascontiguousarray(x, np.float32)
    b = x.view(np.uint32).astype(np.uint64)
    shift = 12
    half = np.uint64(1 << (shift - 1))
    one = np.uint64(1 << shift)
    low = b & (one - np.uint64(1))
    base = b & np.uint64(~((1 << shift) - 1) & 0xFFFFFFFFFFFFFFFF)
    odd = ((b >> np.uint64(shift)) & np.uint64(1)).astype(bool)
    up = (low > half) | ((low == half) & odd)
    out = base + np.where(up, one, np.uint64(0))
    return out.astype(np.uint32).view(np.float32)


def _prep_inputs(inputs):
    """Host-side: transpose features, shard, pre-arrange + f32r-round."""
    f = np.float32
    r = _rnd_f32r
    nf = r(np.asarray(inputs["node_feat"]).T)
    rf = r(np.asarray(inputs["rf_feat"]).T)
    tx = r(np.asarray(inputs["txp_feat"]).T)
    Wf = np.asarray(inputs["Wf"], f)
    We1 = np.asarray(inputs["We1"], f)
    We2 = np.asarray(inputs["We2"], f)
    Wc = np.asarray(inputs["Wc"], f)
    base = {
        "nfT": nf, "rfT": rf, "txT": tx,
        "Wb": r(inputs["Wb"]), "Wr": r(inputs["Wr"]), "Wt": r(inputs["Wt"]),
        "bbT": np.asarray(inputs["bb"], f).reshape(64, 1).copy(),
        "brT": np.asarray(inputs["br"], f).reshape(64, 1).copy(),
        "btT": np.asarray(inputs["bt"], f).reshape(64, 1).copy(),
        "WfB": r(Wf[0:64]), "WfR": r(Wf[64:128]), "WfX": r(Wf[128:192]),
        "bfT": np.asarray(inputs["bf"], f).reshape(HID, 1).copy(),
        "W1a": r(We1[:HID] - We1[HID:]), "W1b": r(We1[HID:]),
        "be1": r(np.asarray(inputs["be1"], f).reshape(1, DGC)),
        "W2a": r(We2[:DGC] - We2[DGC:]), "W2b": r(We2[DGC:]),
        "be2": r(np.asarray(inputs["be2"], f).reshape(1, DGC)),
        "WcA": r(Wc[:DGC]), "WcB": r(Wc[DGC:]),
        "bc": r(np.asarray(inputs["bc"], f).reshape(1, NCLS)),
    }
    in_maps = []
    for c in range(NCORES):
        sl = slice(c * SHARD, (c + 1) * SHARD)
        m = dict(base)
        m["salt"] = np.zeros((1, BUILD_SALT), f)
        m["nfTs"] = np.ascontiguousarray(nf[:, sl])
        m["rfTs"] = np.ascontiguousarray(rf[:, sl])
        m["txTs"] = np.ascontiguousarray(tx[:, sl])
        in_maps.append(m)
    return in_maps


def kernel(**inputs):
    from concourse.bass_utils import run_bass_kernel_spmd

    if "nc" not in _CACHE:
        _CACHE["nc"] = _build()
    nc = _CACHE["nc"]
    in_maps = _prep_inputs(inputs)
    res = run_bass_kernel_spmd(nc, in_maps, core_ids=list(range(NCORES)))
    outs = [res.results[c]["out"] for c in range(NCORES)]
    return np.concatenate(outs, axis=0).astype(np.float32)


if __name__ == "__main__":
    import reference

    ins = {k: np.asarray(v) for k, v in reference.setup_inputs().items()}
    got = kernel(**ins)
    exp = np.asarray(reference.reference(**ins))
    err = np.abs(got - exp)
    print("max abs err:", err.max(), "rel:", err.max() / np.abs(exp).max())
